# revision 14
# baseline (speedup 1.0000x reference)
"""GCN forward (4x GCNConv + linear head) on 8 Trainium2 NeuronCores.

Sharding: nodes are partitioned across the 8 cores (2048 rows each).

Transpose-free layout cycle: the aggregation contracts over edge slots
(gathered rows stationary, scatter matrix moving) and lands feature-major
in PSUM; the dense GEMM contracts over features (aggregated tile
stationary, natural-layout weights moving) and lands node-major, which is
exactly the layout the AllGather and the next layer's row gather need.
Layers 1-3 aggregate first (A_hat h) W; layer 4 runs its GEMM first
(aggregating at width 2048 instead of 4096); the head GEMM consumes the
layer-4 aggregation (feature-major) directly.

Weights stream once per layer (W3 twice, once per node half); W4 is held
resident in a phase-scoped pool that reuses SBUF released by the earlier
phases.  Bias is applied with one extra matmul per accumulation group
(lhsT = first-row-ones, rhs = bias row), so node-major outputs need no
per-free-element bias pass.  The per-tile gather slots are sorted by
source row for monotone DMA addresses; the normalization coefficients and
edge->tile assignment are precomputed on the host.
"""

import os

import numpy as np
import ml_dtypes

try:  # persistent compilation cache: skip walrus recompile across processes
    import jax
    jax.config.update("jax_compilation_cache_dir",
                      os.path.expanduser("~/.cache/jax_gcn_kernel"))
    jax.config.update("jax_persistent_cache_min_compile_time_secs", 10)
except Exception:
    pass

import concourse.bass as bass
import concourse.mybir as mybir
import concourse.tile as tile
from concourse import bacc
from concourse.bass_utils import run_bass_kernel_spmd

N = 16384
E = 65536
NCORES = 8
NL = N // NCORES            # 2048 nodes per core
NT = NL // 128              # 16 dst tiles per core
DIMS = [512, 1024, 2048, 4096, 2048]
C = 1000
CPAD = 1024                 # output classes padded to a multiple of 128

BF = mybir.dt.bfloat16
F32 = mybir.dt.float32
I16 = mybir.dt.int16
RELU = mybir.ActivationFunctionType.Relu
COPY = mybir.ActivationFunctionType.Copy
NPBF = ml_dtypes.bfloat16

_CACHE = {}


# ----------------------------------------------------------------------------
# Device program
# ----------------------------------------------------------------------------

def _build(chunks, cnt_t, dep_t):
    nc = bacc.Bacc("TRN2", target_bir_lowering=False, num_devices=NCORES)

    # ---- kernel I/O ----
    x_bf = nc.dram_tensor("x_bf", [N, DIMS[0]], BF, kind="ExternalInput")
    idx_d = nc.dram_tensor("idx", [128, NT, chunks * 8], I16, kind="ExternalInput")
    s_d = nc.dram_tensor("smat", [128, NT, chunks, 128], BF, kind="ExternalInput")
    ones_d = nc.dram_tensor("onesrow", [128, 512], BF, kind="ExternalInput")
    w1_d = nc.dram_tensor("w1n", [128, 4, 1024], BF, kind="ExternalInput")
    w2_d = nc.dram_tensor("w2n", [128, 8, 2048], BF, kind="ExternalInput")
    w3_d = nc.dram_tensor("w3b", [128, 32, 16, 128], BF, kind="ExternalInput")
    w4_d = nc.dram_tensor("w4n", [128, 32, 2048], BF, kind="ExternalInput")
    wo_d = nc.dram_tensor("won", [128, 16, CPAD], BF, kind="ExternalInput")
    brow1_d = nc.dram_tensor("brow1", [128, 1024], BF, kind="ExternalInput")
    brow2_d = nc.dram_tensor("brow2", [128, 2048], BF, kind="ExternalInput")
    browo_d = nc.dram_tensor("browo", [128, CPAD], BF, kind="ExternalInput")
    b3c_d = nc.dram_tensor("b3c", [128, 32], F32, kind="ExternalInput")
    b4c_d = nc.dram_tensor("b4c", [128, 16], F32, kind="ExternalInput")
    out_d = nc.dram_tensor("out_nm", [NL, CPAD], F32, kind="ExternalOutput")

    # ---- internal DRAM ----
    h2_nm = nc.dram_tensor("h2_nm", [NL, DIMS[1]], BF)
    h2_full = nc.dram_tensor("h2_full", [N, DIMS[1]], BF, addr_space="Shared")
    h3_nm = nc.dram_tensor("h3_nm", [NL, DIMS[2]], BF)
    h3_full = nc.dram_tensor("h3_full", [N, DIMS[2]], BF, addr_space="Shared")
    h4T = nc.dram_tensor("h4T", [NT, 128, 32, 128], BF)   # [t][feat_p][kb][node]
    m4_nm = nc.dram_tensor("m4_nm", [NL, DIMS[4]], BF)
    m4_full = nc.dram_tensor("m4_full", [N, DIMS[4]], BF, addr_space="Shared")

    rg = [list(range(NCORES))]

    with tile.TileContext(nc) as tc:
        with (
            tc.tile_pool(name="const", bufs=1) as p_const,
            tc.tile_pool(name="brow", bufs=1) as p_brow,
            tc.tile_pool(name="bcol", bufs=2) as p_bcol,
            tc.tile_pool(name="ht", bufs=2) as p_ht,
            tc.tile_pool(name="w3s", bufs=2) as p_w3s,
            tc.tile_pool(name="gath", bufs=7) as p_gath,
            tc.tile_pool(name="aggps", bufs=3, space="PSUM") as p_agg,
            tc.tile_pool(name="warmps", bufs=1, space="PSUM") as p_warm,
            tc.tile_pool(name="gemmps", bufs=2, space="PSUM") as p_gemm,
        ):
            # ---- constants ----
            idx_sb = p_const.tile([128, NT, chunks * 8], I16)
            nc.sync.dma_start(out=idx_sb[:], in_=idx_d[:])
            s_sb = p_const.tile([128, NT, chunks, 128], BF)
            nc.sync.dma_start(out=s_sb[:], in_=s_d[:])
            ones_sb = p_const.tile([128, 512], BF)
            nc.sync.dma_start(out=ones_sb[:], in_=ones_d[:])

            def tile_groups(t, split):
                """Chunk runs with equal AG-block dep, capped at 2 chunks."""
                nct = cnt_t[t]
                groups = []
                c0 = 0
                for c in range(1, nct + 1):
                    if (c == nct or (split and dep_t[t][c] != dep_t[t][c0])
                            or c - c0 == 2):
                        groups.append((c0, c, dep_t[t][c0] if split else 3))
                        c0 = c
                return groups

            def emit_gather(src_full, fa, t, grp, gtiles):
                (c0, c1, dep) = grp
                ng = c1 - c0
                rows = (dep + 1) * (N // 4)
                gt = p_gath.tile([128, ng, fa], BF, tag="gath")
                nc.gpsimd.dma_gather(
                    gt[:], src_full.ap()[0:rows, :],
                    idx_sb[:, t, c0 * 8:c1 * 8],
                    ng * 128, ng * 128, fa,
                )
                for c in range(c0, c1):
                    gtiles[(t, c)] = (gt, c - c0)

            def agg_tile_mms(fa, t, tt, aggT, gtiles, bcol, warm):
                nct = cnt_t[t]
                for g in range(fa // 512):
                    aps = p_agg.tile([128, 4, 128], F32, tag="aggps")
                    for q in range(4):
                        fb = g * 4 + q
                        for c in range(nct):
                            gt, ci = gtiles[(t, c)]
                            nc.tensor.matmul(
                                out=aps[:, q, :],
                                lhsT=gt[:, ci, fb * 128:(fb + 1) * 128],
                                rhs=s_sb[:, t, c, :],
                                start=(c == 0), stop=(c == nct - 1),
                            )
                    if bcol is None:
                        nc.vector.tensor_copy(
                            out=aggT[:, g * 4:(g + 1) * 4,
                                     tt * 128:(tt + 1) * 128],
                            in_=aps[:],
                        )
                    else:
                        for q in range(4):
                            fb = g * 4 + q
                            nc.vector.tensor_scalar(
                                out=aggT[:, fb, tt * 128:(tt + 1) * 128],
                                in0=aps[:, q, :],
                                scalar1=bcol[:, fb:fb + 1], scalar2=0.0,
                                op0=mybir.AluOpType.add,
                                op1=mybir.AluOpType.max,
                            )
                if warm:
                    # keep the PE HAM activity window busy through the
                    # gather-bound stretches so matmuls stay at 2.4 GHz
                    wp = p_warm.tile([128, 512], F32, tag="warmps")
                    for _ in range(6):
                        nc.tensor.matmul(out=wp[:], lhsT=ones_sb[:, 0:128],
                                         rhs=ones_sb[:], start=True, stop=True)

            def agg_half(src_full, fa, half, aggT, bcol=None, split=True,
                         warm=False):
                """Aggregate dst tiles [half*8, half*8+8) at width fa into
                aggT [128, fa//128, 1024], feature-major bf16.  With bcol,
                applies relu(x + b) during the PSUM evacuation (layer-4).

                Slots are host-sorted by source row, so each chunk run only
                needs an AG-block prefix of src_full (split=True).  The
                first two tiles' gathers are emitted wave-interleaved so
                the in-order gpsimd queue can issue their early chunks
                while later AG quarters are still in flight.
                """
                gtiles = {}
                t0, t1 = half * 8, half * 8 + 1
                g0, g1 = tile_groups(t0, split), tile_groups(t1, split)
                for w in range(max(len(g0), len(g1))):
                    if w < len(g0):
                        emit_gather(src_full, fa, t0, g0[w], gtiles)
                    if w < len(g1):
                        emit_gather(src_full, fa, t1, g1[w], gtiles)
                agg_tile_mms(fa, t0, 0, aggT, gtiles, bcol, warm)
                agg_tile_mms(fa, t1, 1, aggT, gtiles, bcol, warm)
                for tt in range(2, 8):
                    t = half * 8 + tt
                    for grp in tile_groups(t, split):
                        emit_gather(src_full, fa, t, grp, gtiles)
                    agg_tile_mms(fa, t, tt, aggT, gtiles, bcol, warm)

            def gemm_nm(aggT, half, fa, fo, w_sb, brow, relu, out_dram,
                        out_f32=None, ag=None):
                """Node-major GEMM over one half (8 tiles x 128 nodes):
                out[node, fo] = aggT.T @ W (+ b) with optional relu.

                fo is processed in chunks of 1024 (2 PSUM banks)."""
                nkb = fa // 128
                nhc = max(fo // 1024, 1)
                fc = min(fo, 1024)
                for tt in range(8):
                    row0 = (half * 8 + tt) * 128
                    for hc in range(nhc):
                        gps = p_gemm.tile([128, fc], F32, tag="gemmps")
                        for kb in range(nkb):
                            for cc in range(fc // 512):
                                nc.tensor.matmul(
                                    out=gps[:, cc * 512:(cc + 1) * 512],
                                    lhsT=aggT[:, kb, tt * 128:(tt + 1) * 128],
                                    rhs=w_sb[:, kb,
                                             hc * fc + cc * 512:
                                             hc * fc + (cc + 1) * 512],
                                    start=(kb == 0), stop=False,
                                )
                        for cc in range(fc // 512):
                            nc.tensor.matmul(
                                out=gps[:, cc * 512:(cc + 1) * 512],
                                lhsT=ones_sb[:, 0:128],
                                rhs=brow[:, hc * fc + cc * 512:
                                         hc * fc + (cc + 1) * 512],
                                start=False, stop=True,
                            )
                        if out_f32 is not None:
                            of = out_f32.tile([128, fc], F32, tag="outf")
                            nc.vector.tensor_copy(out=of[:], in_=gps[:])
                            nc.sync.dma_start(
                                out=out_dram.ap()[row0:row0 + 128,
                                                  hc * fc:(hc + 1) * fc],
                                in_=of[:],
                            )
                        else:
                            ht = p_ht.tile([128, fc], BF, tag="ht")
                            if relu:
                                nc.scalar.activation(out=ht[:], in_=gps[:],
                                                     func=RELU)
                            else:
                                nc.vector.tensor_copy(out=ht[:], in_=gps[:])
                            nc.sync.dma_start(
                                out=out_dram.ap()[row0:row0 + 128,
                                                  hc * fc:(hc + 1) * fc],
                                in_=ht[:],
                            )
                    if ag is not None and tt % 4 == 3:
                        ag(half * 2 + tt // 4)

            def gemm_fm3(aggT, half, b3c):
                """Layer-3 GEMM, feature-major out: h4T = relu(W3.T @ aggT + b3).

                lhsT = W3 blocks (streamed once per half), rhs = aggT."""
                for mb in range(32):
                    w3t = p_w3s.tile([128, 16, 128], BF, tag="w3s")
                    nc.sync.dma_start(out=w3t[:], in_=w3_d[:, mb, :, :])
                    gps = p_gemm.tile([128, 1024], F32, tag="gemmps")
                    for kb in range(16):
                        for cc in range(2):
                            nc.tensor.matmul(
                                out=gps[:, cc * 512:(cc + 1) * 512],
                                lhsT=w3t[:, kb, :],
                                rhs=aggT[:, kb, cc * 512:(cc + 1) * 512],
                                start=(kb == 0), stop=(kb == 15),
                            )
                    ht = p_ht.tile([128, 1024], BF, tag="ht")
                    nc.scalar.activation(out=ht[:], in_=gps[:], func=RELU,
                                         bias=b3c[:, mb:mb + 1])
                    nc.sync.dma_start(
                        out=h4T.ap()[half * 8:(half + 1) * 8, :, mb, :]
                            .rearrange("t p n -> p t n"),
                        in_=ht[:].rearrange("p (t n) -> p t n", t=8),
                    )

            def allgather_rows(nm, full, k):
                rl = NL // 4
                gl = N // 4
                nc.gpsimd.collective_compute(
                    "AllGather", mybir.AluOpType.bypass, replica_groups=rg,
                    ins=[nm.ap()[k * rl:(k + 1) * rl, :].opt()],
                    outs=[full.ap()[k * gl:(k + 1) * gl, :].opt()],
                )

            # ================= layers 1-3 =================
            with (
                tc.tile_pool(name="wres", bufs=1) as p_wres,
                tc.tile_pool(name="aggT_a", bufs=2) as p_aggT,
            ):
                # ---- layer 1 ----
                w_sb = p_wres.tile([128, 4, 1024], BF, tag="wres")
                nc.sync.dma_start(out=w_sb[:], in_=w1_d[:])
                brow = p_brow.tile([128, 1024], BF, tag="brow")
                nc.sync.dma_start(out=brow[:], in_=brow1_d[:])
                aggT0 = p_aggT.tile([128, 4, 1024], BF, tag="aggT")
                agg_half(x_bf, DIMS[0], 0, aggT0, split=False, warm=True)
                aggT1 = p_aggT.tile([128, 4, 1024], BF, tag="aggT")
                agg_half(x_bf, DIMS[0], 1, aggT1, split=False, warm=True)
                gemm_nm(aggT0, 0, DIMS[0], DIMS[1], w_sb, brow, True, h2_nm,
                        ag=lambda k: allgather_rows(h2_nm, h2_full, k))
                gemm_nm(aggT1, 1, DIMS[0], DIMS[1], w_sb, brow, True, h2_nm,
                        ag=lambda k: allgather_rows(h2_nm, h2_full, k))

                # ---- layer 2 ----
                w_sb = p_wres.tile([128, 8, 2048], BF, tag="wres")
                nc.sync.dma_start(out=w_sb[:], in_=w2_d[:])
                brow = p_brow.tile([128, 2048], BF, tag="brow")
                nc.sync.dma_start(out=brow[:], in_=brow2_d[:])
                aggT0 = p_aggT.tile([128, 8, 1024], BF, tag="aggT")
                agg_half(h2_full, DIMS[1], 0, aggT0, warm=True)
                aggT1 = p_aggT.tile([128, 8, 1024], BF, tag="aggT")
                agg_half(h2_full, DIMS[1], 1, aggT1, warm=True)
                gemm_nm(aggT0, 0, DIMS[1], DIMS[2], w_sb, brow, True, h3_nm,
                        ag=lambda k: allgather_rows(h3_nm, h3_full, k))
                gemm_nm(aggT1, 1, DIMS[1], DIMS[2], w_sb, brow, True, h3_nm,
                        ag=lambda k: allgather_rows(h3_nm, h3_full, k))

                # ---- layer 3 (feature-major out, no AG) ----
                b3c = p_bcol.tile([128, 32], F32, tag="bcol")
                nc.sync.dma_start(out=b3c[:], in_=b3c_d[:])
                aggT0 = p_aggT.tile([128, 16, 1024], BF, tag="aggT")
                agg_half(h3_full, DIMS[2], 0, aggT0)
                gemm_fm3(aggT0, 0, b3c)
                aggT1 = p_aggT.tile([128, 16, 1024], BF, tag="aggT")
                agg_half(h3_full, DIMS[2], 1, aggT1)
                gemm_fm3(aggT1, 1, b3c)

            # ================= layer 4 GEMM: m4 = h4 @ W4 =================
            # Two tile-half passes x two fo-half passes; W4 quarter tiles
            # (bufs=3) stream with prefetch; each tile's m4 row completes
            # within its pass so the quarter AllGathers start early.
            with (
                tc.tile_pool(name="w4", bufs=3) as p_w4,
                tc.tile_pool(name="h4t", bufs=2) as p_h4t,
            ):
                for tq in range(2):
                    for hf in range(2):
                        wq = []
                        for cc in range(2):
                            w4c = p_w4.tile([128, 32, 512], BF, tag="w4")
                            nc.sync.dma_start(
                                out=w4c[:],
                                in_=w4_d[:, :, (hf * 2 + cc) * 512:
                                         (hf * 2 + cc + 1) * 512])
                            wq.append(w4c)
                        for t in range(tq * 8, tq * 8 + 8):
                            lt = p_h4t.tile([128, 32, 128], BF, tag="h4t")
                            nc.sync.dma_start(out=lt[:],
                                              in_=h4T.ap()[t, :, :, :])
                            gps = p_gemm.tile([128, 1024], F32, tag="gemmps")
                            for kb in range(32):
                                for cc in range(2):
                                    nc.tensor.matmul(
                                        out=gps[:, cc * 512:(cc + 1) * 512],
                                        lhsT=lt[:, kb, :],
                                        rhs=wq[cc][:, kb, :],
                                        start=(kb == 0), stop=(kb == 31),
                                    )
                            mt = p_ht.tile([128, 1024], BF, tag="ht")
                            nc.vector.tensor_copy(out=mt[:], in_=gps[:])
                            nc.sync.dma_start(
                                out=m4_nm.ap()[t * 128:(t + 1) * 128,
                                               hf * 1024:(hf + 1) * 1024],
                                in_=mt[:],
                            )
                            if hf == 1 and t % 4 == 3:
                                allgather_rows(m4_nm, m4_full, t // 4)

            # ======== layer 4 aggregation + bias + relu, head GEMM ========
            with (
                tc.tile_pool(name="aggT_b", bufs=2) as p_aggT,
                tc.tile_pool(name="whead", bufs=1) as p_whead,
                tc.tile_pool(name="outf", bufs=2) as p_outf,
            ):
                wo_sb = p_whead.tile([128, 16, CPAD], BF)
                nc.sync.dma_start(out=wo_sb[:], in_=wo_d[:])
                browo = p_brow.tile([128, CPAD], BF, tag="brow")
                nc.sync.dma_start(out=browo[:], in_=browo_d[:])
                b4c = p_bcol.tile([128, 16], F32, tag="bcol")
                nc.sync.dma_start(out=b4c[:], in_=b4c_d[:])

                aggT0 = p_aggT.tile([128, 16, 1024], BF, tag="aggT")
                agg_half(m4_full, DIMS[4], 0, aggT0, bcol=b4c)
                gemm_nm(aggT0, 0, DIMS[4], CPAD, wo_sb, browo, False, out_d,
                        out_f32=p_outf)
                aggT1 = p_aggT.tile([128, 16, 1024], BF, tag="aggT")
                agg_half(m4_full, DIMS[4], 1, aggT1, bcol=b4c)
                gemm_nm(aggT1, 1, DIMS[4], CPAD, wo_sb, browo, False, out_d,
                        out_f32=p_outf)

    nc.compile()
    return nc


# ----------------------------------------------------------------------------
# Host-side preprocessing
# ----------------------------------------------------------------------------

def _balance_tiles(wt):
    """Assign nodes to 128 tiles of exactly 128 nodes, balancing total
    weight; heaviest tiles go to the same tile POSITION on every core so
    the (core-uniform) per-position chunk counts stay minimal.

    Returns perm[new_position] = node."""
    order = np.argsort(-wt, kind="stable")
    nbins = 128
    bins = [[] for _ in range(nbins)]
    bw = np.zeros(nbins, np.int64)
    bn = np.zeros(nbins, np.int64)
    for n in order:
        open_b = bn < 128
        cand = np.where(open_b)[0]
        b = cand[np.argmin(bw[cand])]
        bins[b].append(n)
        bw[b] += wt[n]
        bn[b] += 1
    # local refinement: swap nodes between heaviest/lightest bins
    for _ in range(256):
        hi, lo = int(np.argmax(bw)), int(np.argmin(bw))
        if bw[hi] - bw[lo] <= 1:
            break
        d = bw[hi] - bw[lo]
        ah, al = np.asarray(bins[hi]), np.asarray(bins[lo])
        diff = wt[ah][:, None] - wt[al][None, :]
        good = (diff > 0) & (diff <= d)
        if not good.any():
            break
        # pick the swap closest to halving the imbalance
        score = np.where(good, -np.abs(diff - d // 2), -10**9)
        ii, jj = np.unravel_index(np.argmax(score), diff.shape)
        ni, nj = int(ah[ii]), int(al[jj])
        bins[hi][int(ii)], bins[lo][int(jj)] = nj, ni
        delta = wt[ni] - wt[nj]
        bw[hi] -= delta
        bw[lo] += delta
    # heaviest bins to highest tile position on each core (round-robin)
    bin_order = np.argsort(bw)                # light..heavy
    perm = np.zeros(N, np.int64)
    for i, b in enumerate(bin_order):
        t = i // NCORES                        # tile position 0..15
        r = i % NCORES                         # core
        g = r * NT + t
        perm[g * 128:(g + 1) * 128] = bins[b]
    return perm


def _prep_graph(edge_src, edge_dst, edge_weight):
    src = np.asarray(edge_src).astype(np.int64)
    dst = np.asarray(edge_dst).astype(np.int64)
    ew = np.asarray(edge_weight).astype(np.float64)

    deg = np.bincount(dst, weights=ew, minlength=N) + 1.0
    dinv = 1.0 / np.sqrt(deg)
    norm = (dinv[src] * ew * dinv[dst]).astype(np.float32)
    selfc = (dinv * dinv).astype(np.float32)

    # balance in-degree(+self) across tiles with a global permutation:
    # position p holds node perm[p]; ipos[node] = position
    wt = np.bincount(dst, minlength=N).astype(np.int64) + 1
    perm = _balance_tiles(wt)
    ipos = np.zeros(N, np.int64)
    ipos[perm] = np.arange(N)

    # combined edge + self-loop lists, in position space
    nodes = np.arange(N)
    asrc = ipos[np.concatenate([src, nodes])]
    adst = ipos[np.concatenate([dst, nodes])]
    aval = np.concatenate([norm, selfc])

    # AG'd tensors land as row blocks: quarter-split AG block k holds rank
    # r's shard rows [k*NL/4,(k+1)*NL/4) at full rows k*N/4 + r*NL/4 + ...
    def remap(n):
        r = n // NL
        l = n % NL
        blk = l // (NL // 4)
        return (blk * (N // 4) + r * (NL // 4) + l % (NL // 4)).astype(np.int64)

    rsrc = remap(asrc)

    gtile = adst // 128                     # global dst tile 0..127
    counts = np.bincount(gtile, minlength=128)
    # per tile POSITION (max over cores) chunk count, core-uniform program
    cnt_rt = counts.reshape(NCORES, NT)
    cnt_t = tuple(int(np.ceil(cnt_rt[:, t].max() / 128.0)) for t in range(NT))
    chunks = max(cnt_t)
    cap = chunks * 128

    # sort slots within each tile by remapped source row (monotone DMA,
    # and chunk c covers an AG-block staircase for dependency splitting)
    order = np.lexsort((rsrc, gtile))
    starts = np.zeros(128, np.int64)
    starts[1:] = np.cumsum(counts)[:-1]
    pos_sorted = np.arange(len(asrc)) - starts[gtile[order]]

    idx_all = np.zeros((128, cap), np.int16)
    val_all = np.zeros((128, cap), np.float32)
    m_all = np.zeros((128, cap), np.int64)
    hi_all = np.zeros((128, cap), np.int64)
    idx_all[gtile[order], pos_sorted] = rsrc[order].astype(np.int16)
    val_all[gtile[order], pos_sorted] = aval[order]
    m_all[gtile[order], pos_sorted] = adst[order] - gtile[order] * 128
    hi_all[gtile[order], pos_sorted] = rsrc[order]

    # AG-block dependency per (tile position, chunk): which prefix of
    # h_full each chunk's gather needs, maxed over cores
    dep_t = []
    for t in range(NT):
        deps = []
        for c in range(cnt_t[t]):
            hi = hi_all.reshape(NCORES, NT, cap)[:, t, c * 128:(c + 1) * 128]
            deps.append(int(hi.max() // (N // 4)))
        dep_t.append(tuple(deps))
    dep_t = tuple(dep_t)

    # dense scatter matrices S[tile, chunk, k, m]
    s_dense = np.zeros((128, chunks, 128, 128), np.float32)
    ttg = np.repeat(np.arange(128), cap)
    pp = np.tile(np.arange(cap), 128)
    s_dense[ttg, pp // 128, pp % 128, m_all.reshape(-1)] = val_all.reshape(-1)
    # padding slots (val 0) may alias dst 0; they contribute 0 regardless.

    # per-core device layouts
    idx_dev = np.zeros((NCORES, 128, NT, chunks * 8), np.int16)
    s_dev = np.zeros((NCORES, 128, NT, chunks, 128), NPBF)
    for r in range(NCORES):
        for t in range(NT):
            g = r * NT + t
            packed = idx_all[g].reshape(-1, 16).T          # [16, chunks*8]
            idx_dev[r, :, t, :] = np.tile(packed, (8, 1))
            s_dev[r, :, t, :, :] = s_dense[g].transpose(1, 0, 2).astype(NPBF)
    return chunks, cnt_t, dep_t, perm, idx_dev, s_dev


def _prep_weights(inputs):
    """Natural (feature-major-contraction) weight layouts."""
    W1 = np.asarray(inputs["W1"], np.float32)
    W2 = np.asarray(inputs["W2"], np.float32)
    W3 = np.asarray(inputs["W3"], np.float32)
    W4 = np.asarray(inputs["W4"], np.float32)
    Wo = np.zeros((DIMS[4], CPAD), np.float32)
    Wo[:, :C] = np.asarray(inputs["Wout"], np.float32)

    def nat(w):   # [fa, fo] -> [128, fa//128, fo]
        fa, fo = w.shape
        return np.ascontiguousarray(
            w.reshape(fa // 128, 128, fo).transpose(1, 0, 2)).astype(NPBF)

    # W3 as lhsT blocks [128, mb, kb, 128]
    w3b = np.ascontiguousarray(
        W3.reshape(16, 128, 32, 128).transpose(1, 2, 0, 3)).astype(NPBF)

    b1 = np.asarray(inputs["b1"], np.float32)
    b2 = np.asarray(inputs["b2"], np.float32)
    b3 = np.asarray(inputs["b3"], np.float32)
    b4 = np.asarray(inputs["b4"], np.float32)
    bo = np.zeros(CPAD, np.float32)
    bo[:C] = np.asarray(inputs["bout"], np.float32)

    def brow(b, n):
        r = np.zeros((128, n), NPBF)
        r[0, :] = b.astype(NPBF)
        return r

    ones = np.zeros((128, 512), NPBF)
    ones[0, :128] = NPBF(1.0)

    return {
        "w1n": nat(W1), "w2n": nat(W2), "w3b": w3b, "w4n": nat(W4),
        "won": nat(Wo),
        "brow1": brow(b1, 1024), "brow2": brow(b2, 2048),
        "browo": brow(bo, CPAD),
        "b3c": np.ascontiguousarray(b3.reshape(32, 128).T),
        "b4c": np.ascontiguousarray(b4.reshape(16, 128).T),
        "onesrow": ones,
    }


def _run(inputs, trace=False, **kw):
    x = np.asarray(inputs["x"], np.float32)
    chunks, cnt_t, dep_t, perm, idx_dev, s_dev = _prep_graph(
        inputs["edge_src"], inputs["edge_dst"], inputs["edge_weight"])
    wmap = _prep_weights(inputs)

    key = (chunks, cnt_t, dep_t)
    if key not in _CACHE:
        _CACHE[key] = _build(chunks, cnt_t, dep_t)
    nc = _CACHE[key]

    # position p holds node perm[p]; x rows land at the quarter-split AG
    # remap of p so layer 1 shares the gather indices of layers 2-4
    pos = np.arange(N)
    rmp = (pos % NL) // (NL // 4) * (N // 4) \
        + (pos // NL) * (NL // 4) + (pos % NL) % (NL // 4)
    x_rm = np.empty_like(x)
    x_rm[rmp] = x[perm]
    x_bf = np.ascontiguousarray(x_rm).astype(NPBF)
    in_maps = []
    for r in range(NCORES):
        m = {"x_bf": x_bf, "idx": idx_dev[r], "smat": s_dev[r], **wmap}
        in_maps.append(m)

    res = run_bass_kernel_spmd(nc, in_maps, core_ids=list(range(NCORES)),
                               trace=trace, **kw)
    dev = np.concatenate(
        [res.results[r]["out_nm"][:, :C] for r in range(NCORES)], axis=0)
    out = np.empty_like(dev)
    out[perm] = dev
    return np.ascontiguousarray(out.astype(np.float32)), res


def kernel(**inputs) -> np.ndarray:
    out, _ = _run(inputs, trace=False)
    return out


# revision 16
# speedup vs baseline: 1.0520x; 1.0520x over previous
"""GCN forward (4x GCNConv + linear head) on 8 Trainium2 NeuronCores.

Sharding: nodes are partitioned across the 8 cores (2048 rows each).

Transpose-free layout cycle: the aggregation contracts over edge slots
(gathered rows stationary, scatter matrix moving) and lands feature-major
in PSUM; the dense GEMM contracts over features (aggregated tile
stationary, natural-layout weights moving) and lands node-major, which is
exactly the layout the AllGather and the next layer's row gather need.
Layers 1-3 aggregate first (A_hat h) W; layer 4 runs its GEMM first
(aggregating at width 2048 instead of 4096); the head GEMM consumes the
layer-4 aggregation (feature-major) directly.

Weights stream once per layer (W3 twice, once per node half); W4 is held
resident in a phase-scoped pool that reuses SBUF released by the earlier
phases.  Bias is applied with one extra matmul per accumulation group
(lhsT = first-row-ones, rhs = bias row), so node-major outputs need no
per-free-element bias pass.  The per-tile gather slots are sorted by
source row for monotone DMA addresses; the normalization coefficients and
edge->tile assignment are precomputed on the host.
"""

import os

import numpy as np
import ml_dtypes

try:  # persistent compilation cache: skip walrus recompile across processes
    import jax
    jax.config.update("jax_compilation_cache_dir",
                      os.path.expanduser("~/.cache/jax_gcn_kernel"))
    jax.config.update("jax_persistent_cache_min_compile_time_secs", 10)
except Exception:
    pass

import concourse.bass as bass
import concourse.mybir as mybir
import concourse.tile as tile
from concourse import bacc
from concourse.bass_utils import run_bass_kernel_spmd

N = 16384
E = 65536
NCORES = 8
NL = N // NCORES            # 2048 nodes per core
NT = NL // 128              # 16 dst tiles per core
DIMS = [512, 1024, 2048, 4096, 2048]
C = 1000
CPAD = 1024                 # output classes padded to a multiple of 128

BF = mybir.dt.bfloat16
F32 = mybir.dt.float32
I16 = mybir.dt.int16
RELU = mybir.ActivationFunctionType.Relu
COPY = mybir.ActivationFunctionType.Copy
NPBF = ml_dtypes.bfloat16

_CACHE = {}


# ----------------------------------------------------------------------------
# Device program
# ----------------------------------------------------------------------------

def _build(chunks, cnt_t, dep_t):
    nc = bacc.Bacc("TRN2", target_bir_lowering=False, num_devices=NCORES)

    # ---- kernel I/O ----
    x_bf = nc.dram_tensor("x_bf", [N, DIMS[0]], BF, kind="ExternalInput")
    idx_d = nc.dram_tensor("idx", [128, NT, chunks * 8], I16, kind="ExternalInput")
    s_d = nc.dram_tensor("smat", [128, NT, chunks, 128], BF, kind="ExternalInput")
    ones_d = nc.dram_tensor("onesrow", [128, 512], BF, kind="ExternalInput")
    w1_d = nc.dram_tensor("w1n", [128, 4, 1024], BF, kind="ExternalInput")
    w2_d = nc.dram_tensor("w2n", [128, 8, 2048], BF, kind="ExternalInput")
    w3_d = nc.dram_tensor("w3b", [128, 32, 16, 128], BF, kind="ExternalInput")
    w4_d = nc.dram_tensor("w4n", [128, 32, 2048], BF, kind="ExternalInput")
    wo_d = nc.dram_tensor("won", [128, 16, CPAD], BF, kind="ExternalInput")
    brow1_d = nc.dram_tensor("brow1", [128, 1024], BF, kind="ExternalInput")
    brow2_d = nc.dram_tensor("brow2", [128, 2048], BF, kind="ExternalInput")
    browo_d = nc.dram_tensor("browo", [128, CPAD], BF, kind="ExternalInput")
    b3c_d = nc.dram_tensor("b3c", [128, 32], F32, kind="ExternalInput")
    b4c_d = nc.dram_tensor("b4c", [128, 16], F32, kind="ExternalInput")
    out_d = nc.dram_tensor("out_nm", [NL, CPAD], F32, kind="ExternalOutput")

    # ---- internal DRAM ----
    h2_nm = nc.dram_tensor("h2_nm", [NL, DIMS[1]], BF)
    h2_full = nc.dram_tensor("h2_full", [N, DIMS[1]], BF, addr_space="Shared")
    h3_nm = nc.dram_tensor("h3_nm", [NL, DIMS[2]], BF)
    h3_full = nc.dram_tensor("h3_full", [N, DIMS[2]], BF, addr_space="Shared")
    h4T = nc.dram_tensor("h4T", [NT, 128, 32, 128], BF)   # [t][feat_p][kb][node]
    m4_nm = nc.dram_tensor("m4_nm", [NL, DIMS[4]], BF)
    m4_full = nc.dram_tensor("m4_full", [N, DIMS[4]], BF, addr_space="Shared")

    rg = [list(range(NCORES))]

    with tile.TileContext(nc) as tc:
        with (
            tc.tile_pool(name="const", bufs=1) as p_const,
            tc.tile_pool(name="brow", bufs=1) as p_brow,
            tc.tile_pool(name="bcol", bufs=2) as p_bcol,
            tc.tile_pool(name="ht", bufs=2) as p_ht,
            tc.tile_pool(name="w3s", bufs=2) as p_w3s,
            tc.tile_pool(name="aggps", bufs=3, space="PSUM") as p_agg,
            tc.tile_pool(name="warmps", bufs=1, space="PSUM") as p_warm,
            tc.tile_pool(name="gemmps", bufs=2, space="PSUM") as p_gemm,
        ):
            # ---- constants ----
            idx_sb = p_const.tile([128, NT, chunks * 8], I16)
            nc.sync.dma_start(out=idx_sb[:], in_=idx_d[:])
            s_sb = p_const.tile([128, NT, chunks, 128], BF)
            nc.sync.dma_start(out=s_sb[:], in_=s_d[:])
            ones_sb = p_const.tile([128, 512], BF)
            nc.sync.dma_start(out=ones_sb[:], in_=ones_d[:])

            def agg_half(src_full, fa, half, aggT, bcol=None, split=True,
                         warm=False):
                """Aggregate dst tiles [half*8, half*8+8) at width fa into
                aggT [128, fa//128, 1024], feature-major bf16.  With bcol,
                applies relu(x + b) during the PSUM evacuation (layer-4).

                Slots are host-sorted by source row, so with split=True
                each chunk run only needs an AG-block prefix of src_full
                and can gather while later AG quarters are in flight.
                """
                for tt in range(8):
                    t = half * 8 + tt
                    nct = cnt_t[t]
                    gath = p_gath.tile([128, chunks, fa], BF, tag="gath")
                    if split:
                        groups = []
                        c0 = 0
                        for c in range(1, nct + 1):
                            if c == nct or dep_t[t][c] != dep_t[t][c0]:
                                groups.append((c0, c, dep_t[t][c0]))
                                c0 = c
                    else:
                        groups = [(0, nct, 3)]
                    for (c0, c1, dep) in groups:
                        ng = c1 - c0
                        rows = (dep + 1) * (N // 4)
                        nc.gpsimd.dma_gather(
                            gath[:, c0:c1, :], src_full.ap()[0:rows, :],
                            idx_sb[:, t, c0 * 8:c1 * 8],
                            ng * 128, ng * 128, fa,
                        )
                    for g in range(fa // 512):
                        aps = p_agg.tile([128, 4, 128], F32, tag="aggps")
                        for q in range(4):
                            fb = g * 4 + q
                            for c in range(nct):
                                nc.tensor.matmul(
                                    out=aps[:, q, :],
                                    lhsT=gath[:, c, fb * 128:(fb + 1) * 128],
                                    rhs=s_sb[:, t, c, :],
                                    start=(c == 0), stop=(c == nct - 1),
                                )
                        if bcol is None:
                            nc.vector.tensor_copy(
                                out=aggT[:, g * 4:(g + 1) * 4,
                                         tt * 128:(tt + 1) * 128],
                                in_=aps[:],
                            )
                        else:
                            for q in range(4):
                                fb = g * 4 + q
                                nc.vector.tensor_scalar(
                                    out=aggT[:, fb, tt * 128:(tt + 1) * 128],
                                    in0=aps[:, q, :],
                                    scalar1=bcol[:, fb:fb + 1], scalar2=0.0,
                                    op0=mybir.AluOpType.add,
                                    op1=mybir.AluOpType.max,
                                )
                    if warm:
                        # keep the PE HAM activity window busy through the
                        # gather-bound stretches so matmuls stay warm
                        wp = p_warm.tile([128, 512], F32, tag="warmps")
                        for _ in range(6):
                            nc.tensor.matmul(out=wp[:], lhsT=ones_sb[:, 0:128],
                                             rhs=ones_sb[:], start=True,
                                             stop=True)

            def gemm_nm(aggT, half, fa, fo, w_sb, brow, relu, out_dram,
                        out_f32=None, ag=None):
                """Node-major GEMM over one half (8 tiles x 128 nodes):
                out[node, fo] = aggT.T @ W (+ b) with optional relu.

                fo is processed in chunks of 1024 (2 PSUM banks)."""
                nkb = fa // 128
                nhc = max(fo // 1024, 1)
                fc = min(fo, 1024)
                for tt in range(8):
                    row0 = (half * 8 + tt) * 128
                    for hc in range(nhc):
                        gps = p_gemm.tile([128, fc], F32, tag="gemmps")
                        for kb in range(nkb):
                            for cc in range(fc // 512):
                                nc.tensor.matmul(
                                    out=gps[:, cc * 512:(cc + 1) * 512],
                                    lhsT=aggT[:, kb, tt * 128:(tt + 1) * 128],
                                    rhs=w_sb[:, kb,
                                             hc * fc + cc * 512:
                                             hc * fc + (cc + 1) * 512],
                                    start=(kb == 0), stop=False,
                                )
                        for cc in range(fc // 512):
                            nc.tensor.matmul(
                                out=gps[:, cc * 512:(cc + 1) * 512],
                                lhsT=ones_sb[:, 0:128],
                                rhs=brow[:, hc * fc + cc * 512:
                                         hc * fc + (cc + 1) * 512],
                                start=False, stop=True,
                            )
                        if out_f32 is not None:
                            of = out_f32.tile([128, fc], F32, tag="outf")
                            nc.vector.tensor_copy(out=of[:], in_=gps[:])
                            nc.scalar.dma_start(
                                out=out_dram.ap()[row0:row0 + 128,
                                                  hc * fc:(hc + 1) * fc],
                                in_=of[:],
                            )
                        else:
                            ht = p_ht.tile([128, fc], BF, tag="ht")
                            if relu:
                                nc.scalar.activation(out=ht[:], in_=gps[:],
                                                     func=RELU)
                            else:
                                nc.vector.tensor_copy(out=ht[:], in_=gps[:])
                            nc.scalar.dma_start(
                                out=out_dram.ap()[row0:row0 + 128,
                                                  hc * fc:(hc + 1) * fc],
                                in_=ht[:],
                            )
                    if ag is not None and tt % 4 == 3:
                        ag(half * 2 + tt // 4)

            def gemm_fm3(aggT, half, b3c):
                """Layer-3 GEMM, feature-major out: h4T = relu(W3.T @ aggT + b3).

                lhsT = W3 blocks (streamed once per half), rhs = aggT."""
                for mb in range(32):
                    w3t = p_w3s.tile([128, 16, 128], BF, tag="w3s")
                    nc.sync.dma_start(out=w3t[:], in_=w3_d[:, mb, :, :])
                    gps = p_gemm.tile([128, 1024], F32, tag="gemmps")
                    for kb in range(16):
                        for cc in range(2):
                            nc.tensor.matmul(
                                out=gps[:, cc * 512:(cc + 1) * 512],
                                lhsT=w3t[:, kb, :],
                                rhs=aggT[:, kb, cc * 512:(cc + 1) * 512],
                                start=(kb == 0), stop=(kb == 15),
                            )
                    ht = p_ht.tile([128, 1024], BF, tag="ht")
                    nc.scalar.activation(out=ht[:], in_=gps[:], func=RELU,
                                         bias=b3c[:, mb:mb + 1])
                    nc.scalar.dma_start(
                        out=h4T.ap()[half * 8:(half + 1) * 8, :, mb, :]
                            .rearrange("t p n -> p t n"),
                        in_=ht[:].rearrange("p (t n) -> p t n", t=8),
                    )

            def allgather_rows(nm, full, k):
                rl = NL // 4
                gl = N // 4
                nc.gpsimd.collective_compute(
                    "AllGather", mybir.AluOpType.bypass, replica_groups=rg,
                    ins=[nm.ap()[k * rl:(k + 1) * rl, :].opt()],
                    outs=[full.ap()[k * gl:(k + 1) * gl, :].opt()],
                )

            # ================= layers 1-3 =================
            with (
                tc.tile_pool(name="wres", bufs=1) as p_wres,
                tc.tile_pool(name="gath_a", bufs=2) as p_gath,
                tc.tile_pool(name="aggT_a", bufs=2) as p_aggT,
            ):
                # ---- layer 1 ----
                w_sb = p_wres.tile([128, 4, 1024], BF, tag="wres")
                nc.sync.dma_start(out=w_sb[:], in_=w1_d[:])
                brow = p_brow.tile([128, 1024], BF, tag="brow")
                nc.sync.dma_start(out=brow[:], in_=brow1_d[:])
                aggT0 = p_aggT.tile([128, 4, 1024], BF, tag="aggT")
                agg_half(x_bf, DIMS[0], 0, aggT0, split=False, warm=True)
                aggT1 = p_aggT.tile([128, 4, 1024], BF, tag="aggT")
                agg_half(x_bf, DIMS[0], 1, aggT1, split=False, warm=True)
                gemm_nm(aggT0, 0, DIMS[0], DIMS[1], w_sb, brow, True, h2_nm,
                        ag=lambda k: allgather_rows(h2_nm, h2_full, k))
                gemm_nm(aggT1, 1, DIMS[0], DIMS[1], w_sb, brow, True, h2_nm,
                        ag=lambda k: allgather_rows(h2_nm, h2_full, k))

                # ---- layer 2 ----
                w_sb = p_wres.tile([128, 8, 2048], BF, tag="wres")
                nc.sync.dma_start(out=w_sb[:], in_=w2_d[:])
                brow = p_brow.tile([128, 2048], BF, tag="brow")
                nc.sync.dma_start(out=brow[:], in_=brow2_d[:])
                aggT0 = p_aggT.tile([128, 8, 1024], BF, tag="aggT")
                agg_half(h2_full, DIMS[1], 0, aggT0, warm=True)
                aggT1 = p_aggT.tile([128, 8, 1024], BF, tag="aggT")
                agg_half(h2_full, DIMS[1], 1, aggT1, warm=True)
                gemm_nm(aggT0, 0, DIMS[1], DIMS[2], w_sb, brow, True, h3_nm,
                        ag=lambda k: allgather_rows(h3_nm, h3_full, k))
                gemm_nm(aggT1, 1, DIMS[1], DIMS[2], w_sb, brow, True, h3_nm,
                        ag=lambda k: allgather_rows(h3_nm, h3_full, k))

                # ---- layer 3 (feature-major out, no AG) ----
                b3c = p_bcol.tile([128, 32], F32, tag="bcol")
                nc.sync.dma_start(out=b3c[:], in_=b3c_d[:])
                aggT0 = p_aggT.tile([128, 16, 1024], BF, tag="aggT")
                agg_half(h3_full, DIMS[2], 0, aggT0)
                gemm_fm3(aggT0, 0, b3c)
                aggT1 = p_aggT.tile([128, 16, 1024], BF, tag="aggT")
                agg_half(h3_full, DIMS[2], 1, aggT1)
                gemm_fm3(aggT1, 1, b3c)

            # ================= layer 4 GEMM: m4 = h4 @ W4 =================
            with tc.tile_pool(name="w4", bufs=4) as p_w4:
                w4q = []
                for q in range(4):
                    wq = p_w4.tile([128, 32, 512], BF, tag="w4")
                    nc.sync.dma_start(out=wq[:],
                                      in_=w4_d[:, :, q * 512:(q + 1) * 512])
                    w4q.append(wq)
                with tc.tile_pool(name="h4t", bufs=2) as p_h4t:
                    for t in range(NT):
                        lt = p_h4t.tile([128, 32, 128], BF, tag="h4t")
                        nc.sync.dma_start(out=lt[:], in_=h4T.ap()[t, :, :, :])
                        for hf in range(2):
                            gps = p_gemm.tile([128, 1024], F32, tag="gemmps")
                            for kb in range(32):
                                for cc in range(2):
                                    nc.tensor.matmul(
                                        out=gps[:, cc * 512:(cc + 1) * 512],
                                        lhsT=lt[:, kb, :],
                                        rhs=w4q[hf * 2 + cc][:, kb, :],
                                        start=(kb == 0), stop=(kb == 31),
                                    )
                            mt = p_ht.tile([128, 1024], BF, tag="ht")
                            nc.vector.tensor_copy(out=mt[:], in_=gps[:])
                            nc.scalar.dma_start(
                                out=m4_nm.ap()[t * 128:(t + 1) * 128,
                                               hf * 1024:(hf + 1) * 1024],
                                in_=mt[:],
                            )
                        if t % 4 == 3:
                            allgather_rows(m4_nm, m4_full, t // 4)

            # ======== layer 4 aggregation + bias + relu, head GEMM ========
            with (
                tc.tile_pool(name="gath_b", bufs=2) as p_gath,
                tc.tile_pool(name="aggT_b", bufs=2) as p_aggT,
                tc.tile_pool(name="whead", bufs=1) as p_whead,
                tc.tile_pool(name="outf", bufs=2) as p_outf,
            ):
                wo_sb = p_whead.tile([128, 16, CPAD], BF)
                nc.sync.dma_start(out=wo_sb[:], in_=wo_d[:])
                browo = p_brow.tile([128, CPAD], BF, tag="brow")
                nc.sync.dma_start(out=browo[:], in_=browo_d[:])
                b4c = p_bcol.tile([128, 16], F32, tag="bcol")
                nc.sync.dma_start(out=b4c[:], in_=b4c_d[:])

                aggT0 = p_aggT.tile([128, 16, 1024], BF, tag="aggT")
                agg_half(m4_full, DIMS[4], 0, aggT0, bcol=b4c)
                gemm_nm(aggT0, 0, DIMS[4], CPAD, wo_sb, browo, False, out_d,
                        out_f32=p_outf)
                aggT1 = p_aggT.tile([128, 16, 1024], BF, tag="aggT")
                agg_half(m4_full, DIMS[4], 1, aggT1, bcol=b4c)
                gemm_nm(aggT1, 1, DIMS[4], CPAD, wo_sb, browo, False, out_d,
                        out_f32=p_outf)

    nc.compile()
    return nc


# ----------------------------------------------------------------------------
# Host-side preprocessing
# ----------------------------------------------------------------------------

def _balance_tiles(wt):
    """Assign nodes to 128 tiles of exactly 128 nodes, balancing total
    weight; heaviest tiles go to the same tile POSITION on every core so
    the (core-uniform) per-position chunk counts stay minimal.

    Returns perm[new_position] = node."""
    order = np.argsort(-wt, kind="stable")
    nbins = 128
    bins = [[] for _ in range(nbins)]
    bw = np.zeros(nbins, np.int64)
    bn = np.zeros(nbins, np.int64)
    for n in order:
        open_b = bn < 128
        cand = np.where(open_b)[0]
        b = cand[np.argmin(bw[cand])]
        bins[b].append(n)
        bw[b] += wt[n]
        bn[b] += 1
    # local refinement: swap nodes between heaviest/lightest bins
    for _ in range(256):
        hi, lo = int(np.argmax(bw)), int(np.argmin(bw))
        if bw[hi] - bw[lo] <= 1:
            break
        d = bw[hi] - bw[lo]
        ah, al = np.asarray(bins[hi]), np.asarray(bins[lo])
        diff = wt[ah][:, None] - wt[al][None, :]
        good = (diff > 0) & (diff <= d)
        if not good.any():
            break
        # pick the swap closest to halving the imbalance
        score = np.where(good, -np.abs(diff - d // 2), -10**9)
        ii, jj = np.unravel_index(np.argmax(score), diff.shape)
        ni, nj = int(ah[ii]), int(al[jj])
        bins[hi][int(ii)], bins[lo][int(jj)] = nj, ni
        delta = wt[ni] - wt[nj]
        bw[hi] -= delta
        bw[lo] += delta
    # heaviest bins to highest tile position on each core (round-robin)
    bin_order = np.argsort(bw)                # light..heavy
    perm = np.zeros(N, np.int64)
    for i, b in enumerate(bin_order):
        t = i // NCORES                        # tile position 0..15
        r = i % NCORES                         # core
        g = r * NT + t
        perm[g * 128:(g + 1) * 128] = bins[b]
    return perm


def _prep_graph(edge_src, edge_dst, edge_weight):
    src = np.asarray(edge_src).astype(np.int64)
    dst = np.asarray(edge_dst).astype(np.int64)
    ew = np.asarray(edge_weight).astype(np.float64)

    deg = np.bincount(dst, weights=ew, minlength=N) + 1.0
    dinv = 1.0 / np.sqrt(deg)
    norm = (dinv[src] * ew * dinv[dst]).astype(np.float32)
    selfc = (dinv * dinv).astype(np.float32)

    # balance in-degree(+self) across tiles with a global permutation:
    # position p holds node perm[p]; ipos[node] = position
    wt = np.bincount(dst, minlength=N).astype(np.int64) + 1
    perm = _balance_tiles(wt)
    ipos = np.zeros(N, np.int64)
    ipos[perm] = np.arange(N)

    # combined edge + self-loop lists, in position space
    nodes = np.arange(N)
    asrc = ipos[np.concatenate([src, nodes])]
    adst = ipos[np.concatenate([dst, nodes])]
    aval = np.concatenate([norm, selfc])

    # AG'd tensors land as row blocks: quarter-split AG block k holds rank
    # r's shard rows [k*NL/4,(k+1)*NL/4) at full rows k*N/4 + r*NL/4 + ...
    def remap(n):
        r = n // NL
        l = n % NL
        blk = l // (NL // 4)
        return (blk * (N // 4) + r * (NL // 4) + l % (NL // 4)).astype(np.int64)

    rsrc = remap(asrc)

    gtile = adst // 128                     # global dst tile 0..127
    counts = np.bincount(gtile, minlength=128)
    # per tile POSITION (max over cores) chunk count, core-uniform program
    cnt_rt = counts.reshape(NCORES, NT)
    cnt_t = tuple(int(np.ceil(cnt_rt[:, t].max() / 128.0)) for t in range(NT))
    chunks = max(cnt_t)
    cap = chunks * 128

    # sort slots within each tile by remapped source row (monotone DMA,
    # and chunk c covers an AG-block staircase for dependency splitting)
    order = np.lexsort((rsrc, gtile))
    starts = np.zeros(128, np.int64)
    starts[1:] = np.cumsum(counts)[:-1]
    pos_sorted = np.arange(len(asrc)) - starts[gtile[order]]

    idx_all = np.zeros((128, cap), np.int16)
    val_all = np.zeros((128, cap), np.float32)
    m_all = np.zeros((128, cap), np.int64)
    hi_all = np.zeros((128, cap), np.int64)
    idx_all[gtile[order], pos_sorted] = rsrc[order].astype(np.int16)
    val_all[gtile[order], pos_sorted] = aval[order]
    m_all[gtile[order], pos_sorted] = adst[order] - gtile[order] * 128
    hi_all[gtile[order], pos_sorted] = rsrc[order]

    # AG-block dependency per (tile position, chunk): which prefix of
    # h_full each chunk's gather needs, maxed over cores
    dep_t = []
    for t in range(NT):
        deps = []
        for c in range(cnt_t[t]):
            hi = hi_all.reshape(NCORES, NT, cap)[:, t, c * 128:(c + 1) * 128]
            deps.append(int(hi.max() // (N // 4)))
        dep_t.append(tuple(deps))
    dep_t = tuple(dep_t)

    # dense scatter matrices S[tile, chunk, k, m]
    s_dense = np.zeros((128, chunks, 128, 128), np.float32)
    ttg = np.repeat(np.arange(128), cap)
    pp = np.tile(np.arange(cap), 128)
    s_dense[ttg, pp // 128, pp % 128, m_all.reshape(-1)] = val_all.reshape(-1)
    # padding slots (val 0) may alias dst 0; they contribute 0 regardless.

    # per-core device layouts
    idx_dev = np.zeros((NCORES, 128, NT, chunks * 8), np.int16)
    s_dev = np.zeros((NCORES, 128, NT, chunks, 128), NPBF)
    for r in range(NCORES):
        for t in range(NT):
            g = r * NT + t
            packed = idx_all[g].reshape(-1, 16).T          # [16, chunks*8]
            idx_dev[r, :, t, :] = np.tile(packed, (8, 1))
            s_dev[r, :, t, :, :] = s_dense[g].transpose(1, 0, 2).astype(NPBF)
    return chunks, cnt_t, dep_t, perm, idx_dev, s_dev


def _prep_weights(inputs):
    """Natural (feature-major-contraction) weight layouts."""
    W1 = np.asarray(inputs["W1"], np.float32)
    W2 = np.asarray(inputs["W2"], np.float32)
    W3 = np.asarray(inputs["W3"], np.float32)
    W4 = np.asarray(inputs["W4"], np.float32)
    Wo = np.zeros((DIMS[4], CPAD), np.float32)
    Wo[:, :C] = np.asarray(inputs["Wout"], np.float32)

    def nat(w):   # [fa, fo] -> [128, fa//128, fo]
        fa, fo = w.shape
        return np.ascontiguousarray(
            w.reshape(fa // 128, 128, fo).transpose(1, 0, 2)).astype(NPBF)

    # W3 as lhsT blocks [128, mb, kb, 128]
    w3b = np.ascontiguousarray(
        W3.reshape(16, 128, 32, 128).transpose(1, 2, 0, 3)).astype(NPBF)

    b1 = np.asarray(inputs["b1"], np.float32)
    b2 = np.asarray(inputs["b2"], np.float32)
    b3 = np.asarray(inputs["b3"], np.float32)
    b4 = np.asarray(inputs["b4"], np.float32)
    bo = np.zeros(CPAD, np.float32)
    bo[:C] = np.asarray(inputs["bout"], np.float32)

    def brow(b, n):
        r = np.zeros((128, n), NPBF)
        r[0, :] = b.astype(NPBF)
        return r

    ones = np.zeros((128, 512), NPBF)
    ones[0, :128] = NPBF(1.0)

    return {
        "w1n": nat(W1), "w2n": nat(W2), "w3b": w3b, "w4n": nat(W4),
        "won": nat(Wo),
        "brow1": brow(b1, 1024), "brow2": brow(b2, 2048),
        "browo": brow(bo, CPAD),
        "b3c": np.ascontiguousarray(b3.reshape(32, 128).T),
        "b4c": np.ascontiguousarray(b4.reshape(16, 128).T),
        "onesrow": ones,
    }


def _run(inputs, trace=False, **kw):
    x = np.asarray(inputs["x"], np.float32)
    chunks, cnt_t, dep_t, perm, idx_dev, s_dev = _prep_graph(
        inputs["edge_src"], inputs["edge_dst"], inputs["edge_weight"])
    wmap = _prep_weights(inputs)

    key = (chunks, cnt_t, dep_t)
    if key not in _CACHE:
        _CACHE[key] = _build(chunks, cnt_t, dep_t)
    nc = _CACHE[key]

    # position p holds node perm[p]; x rows land at the quarter-split AG
    # remap of p so layer 1 shares the gather indices of layers 2-4
    pos = np.arange(N)
    rmp = (pos % NL) // (NL // 4) * (N // 4) \
        + (pos // NL) * (NL // 4) + (pos % NL) % (NL // 4)
    x_rm = np.empty_like(x)
    x_rm[rmp] = x[perm]
    x_bf = np.ascontiguousarray(x_rm).astype(NPBF)
    in_maps = []
    for r in range(NCORES):
        m = {"x_bf": x_bf, "idx": idx_dev[r], "smat": s_dev[r], **wmap}
        in_maps.append(m)

    res = run_bass_kernel_spmd(nc, in_maps, core_ids=list(range(NCORES)),
                               trace=trace, **kw)
    dev = np.concatenate(
        [res.results[r]["out_nm"][:, :C] for r in range(NCORES)], axis=0)
    out = np.empty_like(dev)
    out[perm] = dev
    return np.ascontiguousarray(out.astype(np.float32)), res


def kernel(**inputs) -> np.ndarray:
    out, _ = _run(inputs, trace=False)
    return out
